# revision 37
# baseline (speedup 1.0000x reference)
"""Distributed multi-head attention kernel for 8 TRN2 NeuronCores.

Reference computation (per batch b):
    q = x @ wq.T ; k = x @ wk.T ; v = x @ wv.T          (8 heads x 64)
    attn = softmax(q k^T / sqrt(64)) ; o = attn @ v
    y = concat_heads(o) @ wproj.T

Sharding (per the tensor-parallel hint): core c owns batch b = c // 4
and HEAD PAIR hp = c % 4 (wq/wk/wv split column-wise, wproj split
row-wise).  Each core runs its two heads over ALL 3136 queries and
emits its row-parallel PARTIAL of the output projection,
yp_c[512, 3136]; the unshard step sums the four partials per batch.
ZERO device collectives - the previous ReduceScatter design spent
most of its runtime in ~15 GB/s collectives.

Engine choreography:
  - PE (tensor): qkv projections, qk (two K=64 heads row-tiled into
    array halves via auto tile_position -> they run concurrently),
    av with V STATIONARY (out[hd, q] += v1aug[keys, hd|1].T @ at),
    which both streams efficiently (N=392 bf16) and lands o in the
    [chan, q] layout the projection wants - no PE transposes at all.
    A 65th ones-column in v accumulates the softmax denominator as
    row 64 of the av output.
  - Scalar (Act) + Vector (DVE): exp of both heads of a key-tile in
    ONE FD=784 instruction (the two qk outputs share one PSUM tile),
    alternated between the engines by a tunable quota; DVE runs the
    Schraudolph fast-exp (int16 bits of x*184.665+16256.5 read as
    bf16 ~= e^x), Act runs native Exp.  The av accumulator is staged
    to SBUF with one copy so the PSUM banks recycle immediately; the
    reciprocal (reciprocal_approx_fast, which needs a base-partition-0
    SBUF input) + gpsimd partition-broadcast + multiply run off that
    critical path.
  - k/v/q production is interleaved into the attention loops (just
    in time, like double buffering) so the PE fills exp-paced slack.

All matmuls bf16 with fp32 PSUM accumulation.
"""

import sys

sys.path.insert(0, "/opt/trn_rl_repo")

import ml_dtypes
import numpy as np

B = 2
N = 3136
DIM = 512
HEADS = 8
HD = 64
SCALE = HD**-0.5
N_CORES = 8

CH = 448  # query chunk (7 chunks cover N; 448 f32 fits a PSUM bank)
NCH = N // CH
NKT = 25  # key tiles: 24x128 + 1x64
MT = [(128 * k, min(128, N - 128 * k)) for k in range(NKT)]
KCH = [(o, min(512, N - o)) for o in range(0, N, 512)]  # k/q prod chunks
DEPTH = 3  # av trails qk by DEPTH key-tiles

# Schraudolph fast-exp constants (int16 view of bf16)
EXP_A = 184.66497
EXP_B = 16256.5

# fraction of exp instructions issued to the scalar engine (Act)
ACT_FRAC = 0.52
# fraction of psum->sbuf copies on Act
CPA_FRAC = 0.50

BF16 = ml_dtypes.bfloat16

_CACHE = {}


def _build():
    import concourse.bacc as bacc
    import concourse.mybir as mybir
    import concourse.tile as tile
    from concourse.bass_interp import get_hw_module

    F32 = mybir.dt.float32
    BF = mybir.dt.bfloat16
    I16 = mybir.dt.int16

    nc = bacc.Bacc("TRN2", target_bir_lowering=False, debug=False, num_devices=N_CORES)

    xT_d = nc.dram_tensor("xT", [DIM, N], BF, kind="ExternalInput")
    wq2_d = nc.dram_tensor("wq2", [DIM, 128], BF, kind="ExternalInput")
    wk2_d = nc.dram_tensor("wk2", [DIM, 128], BF, kind="ExternalInput")
    wv2_d = nc.dram_tensor("wv2", [DIM, 128], BF, kind="ExternalInput")
    wp2_d = nc.dram_tensor("wp2", [HD, 2 * 4 * 128], BF, kind="ExternalInput")
    out_d = nc.dram_tensor("out", [DIM, N], BF, kind="ExternalOutput")

    EXP = mybir.ActivationFunctionType.Exp
    COPY = mybir.ActivationFunctionType.Copy
    MULT = mybir.AluOpType.mult
    ADD = mybir.AluOpType.add

    with tile.TileContext(nc) as tc:
        with (
            tc.tile_pool(name="const", bufs=1) as cp,
            tc.tile_pool(name="big", bufs=1) as bp,
            tc.tile_pool(name="attn", bufs=DEPTH + 2) as atp,
            tc.tile_pool(name="norm", bufs=2) as rcp,
            tc.tile_pool(name="psum", bufs=2, space="PSUM") as psp,
        ):
            # ---- activation-table preload: a tiny exp issued first so the
            # ~2.7us ACT_TABLE_LOAD overlaps the input DMAs ----
            warm = cp.tile([1, 16], F32)
            nc.vector.memset(warm[:], 0.0)
            warm2 = cp.tile([1, 16], BF)
            nc.scalar.activation(warm2[:], warm[:], EXP)
            # HAM clock-gate warmup: ~3us of tiny matmuls keep the PE busy
            # through its 4096-cycle activity window while the input DMAs
            # are still in flight, so the real prologue matmuls run warm
            dumm = cp.tile([1, 512], BF)
            nc.vector.memset(dumm[:], 0.0)
            for _ in range(15):
                dw = psp.tile([128, 512], F32, tag="pp", name="dw")
                nc.tensor.matmul(
                    dw[0:16, :], dumm[0:1, 0:16], dumm[0:1, :],
                    start=True, stop=True,
                )

            # ---- weights + inputs ----
            wq2 = cp.tile([128, 4, 128], BF)
            wk2 = cp.tile([128, 4, 128], BF)
            wv2 = cp.tile([128, 4, 128], BF)
            wp2 = cp.tile([HD, 2, 4, 128], BF)
            onesb = cp.tile([1, HD], F32)
            nc.vector.memset(onesb[:], 1.0)

            for t, d in ((wk2, wk2_d), (wq2, wq2_d)):
                for k in range(4):
                    nc.sync.dma_start(t[:, k, :], d[128 * k : 128 * (k + 1), :])
            for k in range(4):
                nc.gpsimd.dma_start(
                    wv2[:, k, :], wv2_d[128 * k : 128 * (k + 1), :]
                )
            nc.gpsimd.dma_start(
                wp2[:].rearrange("p j s c -> p (j s c)"), wp2_d[0:HD, :]
            )

            xT = bp.tile([128, 4, N], BF)
            for k in range(4):
                nc.sync.dma_start(
                    xT[:, k, 0:512], xT_d[128 * k : 128 * (k + 1), 0:512]
                )
            for lo, hi in ((512, 1792), (1792, N)):
                for k in range(4):
                    nc.sync.dma_start(
                        xT[:, k, lo:hi], xT_d[128 * k : 128 * (k + 1), lo:hi]
                    )

            kT = bp.tile([128, N], BF)
            qT = bp.tile([128, N], BF)
            # v1[key, kt, head, hd|1]: 65th column holds ones -> the av
            # matmul accumulates the softmax denominator in out row 64
            v1 = bp.tile([128, NKT, 2, HD + 1], BF)
            nc.vector.memset(v1[:, :, :, HD : HD + 1], 1.0)

            # ---- engine-alternation helpers ----
            exp_acc = [0.0]

            def exp_use_act():
                exp_acc[0] += ACT_FRAC
                if exp_acc[0] >= 1.0:
                    exp_acc[0] -= 1.0
                    return True
                return False

            cp_acc = [0.0]

            def aux_copy(dst, src):
                cp_acc[0] += CPA_FRAC
                if cp_acc[0] >= 1.0:
                    cp_acc[0] -= 1.0
                    nc.scalar.activation(dst, src, COPY)
                else:
                    nc.vector.tensor_copy(dst, src)

            # ---- production units (interleaved with attention) ----
            def prod_k(ci):
                o, n = KCH[ci]
                pp = psp.tile([128, 512], F32, tag="pp", name="pp")
                for k in range(4):
                    nc.tensor.matmul(
                        pp[:, :n],
                        wk2[:, k, :],
                        xT[:, k, o : o + n],
                        start=(k == 0),
                        stop=(k == 3),
                    )
                aux_copy(kT[:, o : o + n], pp[:, :n])

            def prod_q(ci):
                o, n = KCH[ci]
                pp = psp.tile([128, 512], F32, tag="pp", name="pp")
                for k in range(4):
                    nc.tensor.matmul(
                        pp[:, :n],
                        wq2[:, k, :],
                        xT[:, k, o : o + n],
                        start=(k == 0),
                        stop=(k == 3),
                    )
                aux_copy(qT[:, o : o + n], pp[:, :n])

            def prod_v(kt):
                mo, mn = MT[kt]
                pp = psp.tile([128, 512], F32, tag="pp", name="pp")
                for k in range(4):
                    nc.tensor.matmul(
                        pp[:mn, 0:128],
                        xT[:, k, mo : mo + mn],
                        wv2[:, k, :],
                        start=(k == 0),
                        stop=(k == 3),
                    )
                aux_copy(
                    v1[:mn, kt, :, 0:HD],
                    pp[:mn, 0:128].rearrange("p (h c) -> p h c", h=2),
                )

            # production schedule: (ch, slot) -> list of closures
            sched = {}

            def put(ch, slot, fn):
                sched.setdefault((ch, slot), []).append(fn)

            for kt in range(22):  # v tiles 3..24 during ch0
                put(0, kt, (lambda t: lambda: prod_v(t))(kt + 3))
            for c in range(1, 7):  # kT chunks 1..6 during ch0
                put(0, 4 * c - 3, (lambda c_: lambda: prod_k(c_))(c))
            # qT chunks just in time: chunk j first read by query chunk
            # floor(512 j / 392); produce it one chunk earlier
            put(0, 23, lambda: prod_q(1))
            for j, ch in ((2, 1), (3, 2), (4, 3), (5, 4), (6, 5)):
                put(ch, 5, (lambda j_: lambda: prod_q(j_))(j))

            # ---- prologue production: just enough to start ch0 ----
            prod_k(0)
            prod_q(0)
            for kt in range(3):
                prod_v(kt)

            # ---- attention ----
            pending_proj = [None]
            for ch in range(NCH):
                po = psp.tile([128, 2, 512], F32, tag="po", name="po", bufs=1)
                ats = {}
                for slot in range(NKT + DEPTH):
                    if slot < NKT:
                        mo, mn = MT[slot]
                        ps = psp.tile([128, 2, 512], F32, tag="ps", name="ps")
                        for h2 in range(2):
                            hs = slice(64 * h2, 64 * (h2 + 1))
                            nc.tensor.matmul(
                                ps[:mn, h2, :CH],
                                kT[hs, mo : mo + mn],
                                qT[hs, ch * CH : (ch + 1) * CH],
                                start=True,
                                stop=True,
                            )
                        at = atp.tile([128, 2, CH], BF, tag="at", name="at")
                        ats[slot] = at
                        if exp_use_act():
                            nc.scalar.activation(
                                at[:mn, :, :], ps[:mn, :, :CH], EXP
                            )
                        else:
                            nc.vector.tensor_scalar(
                                at[:mn, :, :].bitcast(I16),
                                ps[:mn, :, :CH],
                                EXP_A,
                                EXP_B,
                                MULT,
                                ADD,
                            )
                    kj = slot - DEPTH
                    if kj >= 0:
                        pmo, pmn = MT[kj]
                        pat = ats.pop(kj)
                        for h2 in range(2):
                            nc.tensor.matmul(
                                po[0 : HD + 1, h2, :CH],
                                v1[:pmn, kj, h2, :],
                                pat[:pmn, h2, :],
                                start=(kj == 0),
                                stop=(kj == NKT - 1),
                            )
                    for fn in sched.get((ch, slot), ()):
                        fn()
                    if slot == 2 and pending_proj[0] is not None:
                        pending_proj[0]()
                        pending_proj[0] = None

                # normalize: ONE copy stages the av output (o rows 0-63)
                # into SBUF and a second tiny copy stages the denominator
                # row at base partition 0, releasing the po accumulator;
                # reciprocal / partition-broadcast / multiply then run
                # from SBUF off the po critical path.
                pos = rcp.tile([128, 2, CH], F32, tag="pos", name="pos")
                nc.scalar.activation(pos[0:HD, :, :], po[0:HD, :, :CH], COPY)
                rd = rcp.tile([1, 2, CH], F32, tag="rd", name="rd")
                nc.scalar.activation(rd[0:1, :, :], po[64:65, :, :CH], COPY)
                rc = rcp.tile([1, 2, CH], F32, tag="rc", name="rc")
                nc.vector.reciprocal_approx_fast(
                    out=rc[0:1, :, :], in_=rd[0:1, :, :]
                )
                outn = rcp.tile([HD, 2, CH], BF, tag="on", name="outn")
                for h2 in range(2):
                    bc = psp.tile([128, 512], F32, tag="pp", name="bc")
                    nc.tensor.matmul(
                        bc[0:HD, :CH],
                        onesb[0:1, :],
                        rc[0:1, h2, :],
                        start=True,
                        stop=True,
                    )
                    nc.vector.tensor_tensor(
                        outn[0:HD, h2, :],
                        pos[0:HD, h2, :],
                        bc[0:HD, :CH],
                        MULT,
                    )

                # ---- row-parallel output-projection partial, deferred
                # into the next chunk's early slots (by then the normalize
                # multiply has finished, so the projection matmuls never
                # block the in-order PE queue on it) ----
                def do_proj(ch=ch, outn=outn):
                    for s in range(4):
                        py = psp.tile([128, 512], F32, tag="pp", name="py")
                        for h2 in range(2):
                            nc.tensor.matmul(
                                py[:, :CH],
                                wp2[:, h2, s, :],
                                outn[0:HD, h2, :],
                                start=(h2 == 0),
                                stop=(h2 == 1),
                            )
                        yt = rcp.tile([128, CH], BF, tag="y", name="yt")
                        aux_copy(yt[:, :], py[:, :CH])
                        nc.sync.dma_start(
                            out_d[
                                128 * s : 128 * (s + 1), ch * CH : (ch + 1) * CH
                            ],
                            yt[:, :],
                        )

                pending_proj[0] = do_proj
            pending_proj[0]()

    nc.compile()
    nc.m = get_hw_module(nc.m)
    return nc


def _shard(x, wq, wk, wv, wproj):
    x = np.asarray(x, dtype=np.float32)
    wq = np.asarray(wq, dtype=np.float32)
    wk = np.asarray(wk, dtype=np.float32)
    wv = np.asarray(wv, dtype=np.float32)
    wproj = np.asarray(wproj, dtype=np.float32)

    xT = [np.ascontiguousarray(x[b].T).astype(BF16) for b in range(B)]
    in_maps = []
    for c in range(N_CORES):
        b, hp = c // 4, c % 4
        rows = slice(128 * hp, 128 * (hp + 1))
        wp2 = np.empty((HD, 2, 4, 128), dtype=np.float32)
        for j in range(2):
            for s in range(4):
                wp2[:, j, s, :] = wproj[
                    128 * s : 128 * (s + 1),
                    128 * hp + 64 * j : 128 * hp + 64 * (j + 1),
                ].T
        in_maps.append(
            {
                "xT": xT[b],
                "wq2": np.ascontiguousarray((wq[rows] * SCALE).T).astype(BF16),
                "wk2": np.ascontiguousarray(wk[rows].T).astype(BF16),
                "wv2": np.ascontiguousarray(wv[rows].T).astype(BF16),
                "wp2": np.ascontiguousarray(wp2.reshape(HD, 2 * 4 * 128)).astype(
                    BF16
                ),
            }
        )
    return in_maps


def _unshard(results):
    acc = np.zeros((B, DIM, N), dtype=np.float32)
    for c in range(N_CORES):
        acc[c // 4] += np.asarray(results[c]["out"]).astype(np.float32)
    return np.ascontiguousarray(acc.transpose(0, 2, 1))


def _run(inputs, trace=False):
    from concourse.bass_utils import run_bass_kernel_spmd

    if "nc" not in _CACHE:
        _CACHE["nc"] = _build()
    nc = _CACHE["nc"]
    in_maps = _shard(**inputs)
    res = run_bass_kernel_spmd(
        nc, in_maps, core_ids=list(range(N_CORES)), trace=trace
    )
    return _unshard(res.results), res.exec_time_ns


def kernel(**inputs) -> np.ndarray:
    return _run(inputs, trace=False)[0]


# revision 39
# speedup vs baseline: 1.3895x; 1.3895x over previous
"""Distributed multi-head attention kernel for 8 TRN2 NeuronCores.

Reference computation (per batch b):
    q = x @ wq.T ; k = x @ wk.T ; v = x @ wv.T          (8 heads x 64)
    attn = softmax(q k^T / sqrt(64)) ; o = attn @ v
    y = concat_heads(o) @ wproj.T

Sharding (per the tensor-parallel hint): core c owns batch b = c // 4
and HEAD PAIR hp = c % 4 (wq/wk/wv split column-wise, wproj split
row-wise).  Each core runs its two heads over ALL 3136 queries and
emits its row-parallel PARTIAL of the output projection,
yp_c[512, 3136]; the unshard step sums the four partials per batch.
ZERO device collectives - the previous ReduceScatter design spent
most of its runtime in ~15 GB/s collectives.

Engine choreography:
  - PE (tensor): qkv projections, qk (two K=64 heads row-tiled into
    array halves via auto tile_position -> they run concurrently),
    av with V STATIONARY (out[hd, q] += v1aug[keys, hd|1].T @ at),
    which both streams efficiently (N=392 bf16) and lands o in the
    [chan, q] layout the projection wants - no PE transposes at all.
    A 65th ones-column in v accumulates the softmax denominator as
    row 64 of the av output.
  - Scalar (Act) + Vector (DVE): exp of both heads of a key-tile in
    ONE FD=784 instruction (the two qk outputs share one PSUM tile),
    alternated between the engines by a tunable quota; DVE runs the
    Schraudolph fast-exp (int16 bits of x*184.665+16256.5 read as
    bf16 ~= e^x), Act runs native Exp.  The av accumulator is staged
    to SBUF with one copy so the PSUM banks recycle immediately; the
    reciprocal (reciprocal_approx_fast, which needs a base-partition-0
    SBUF input) + gpsimd partition-broadcast + multiply run off that
    critical path.
  - k/v/q production is interleaved into the attention loops (just
    in time, like double buffering) so the PE fills exp-paced slack.

All matmuls bf16 with fp32 PSUM accumulation.
"""

import sys

sys.path.insert(0, "/opt/trn_rl_repo")

import ml_dtypes
import numpy as np

B = 2
N = 3136
DIM = 512
HEADS = 8
HD = 64
SCALE = HD**-0.5
N_CORES = 8

CH = 448  # query chunk (7 chunks cover N; 448 f32 fits a PSUM bank)
NCH = N // CH
NKT = 25  # key tiles: 24x128 + 1x64
MT = [(128 * k, min(128, N - 128 * k)) for k in range(NKT)]
KCH = [(o, min(512, N - o)) for o in range(0, N, 512)]  # k/q prod chunks
DEPTH = 3  # av trails qk by DEPTH key-tiles

# Schraudolph fast-exp constants (int16 view of bf16)
EXP_A = 184.66497
EXP_B = 16256.5

# fraction of exp instructions issued to the scalar engine (Act)
ACT_FRAC = 0.52
# fraction of psum->sbuf copies on Act
CPA_FRAC = 0.50

BF16 = ml_dtypes.bfloat16

_CACHE = {}


def _build():
    import concourse.bacc as bacc
    import concourse.mybir as mybir
    import concourse.tile as tile
    from concourse.bass_interp import get_hw_module

    F32 = mybir.dt.float32
    BF = mybir.dt.bfloat16
    I16 = mybir.dt.int16

    nc = bacc.Bacc("TRN2", target_bir_lowering=False, debug=False, num_devices=N_CORES)

    xT_d = nc.dram_tensor("xT", [DIM, N], BF, kind="ExternalInput")
    wq2_d = nc.dram_tensor("wq2", [DIM, 128], BF, kind="ExternalInput")
    wk2_d = nc.dram_tensor("wk2", [DIM, 128], BF, kind="ExternalInput")
    wv2_d = nc.dram_tensor("wv2", [DIM, 128], BF, kind="ExternalInput")
    wp2_d = nc.dram_tensor("wp2", [HD, 2 * 4 * 128], BF, kind="ExternalInput")
    out_d = nc.dram_tensor("out", [DIM, N], BF, kind="ExternalOutput")

    EXP = mybir.ActivationFunctionType.Exp
    COPY = mybir.ActivationFunctionType.Copy
    MULT = mybir.AluOpType.mult
    ADD = mybir.AluOpType.add

    with tile.TileContext(nc) as tc:
        with (
            tc.tile_pool(name="const", bufs=1) as cp,
            tc.tile_pool(name="big", bufs=1) as bp,
            tc.tile_pool(name="attn", bufs=DEPTH + 2) as atp,
            tc.tile_pool(name="norm", bufs=2) as rcp,
            tc.tile_pool(name="psum", bufs=2, space="PSUM") as psp,
        ):
            # ---- activation-table preload: a tiny exp issued first so the
            # ~2.7us ACT_TABLE_LOAD overlaps the input DMAs ----
            warm = cp.tile([1, 16], F32)
            nc.vector.memset(warm[:], 0.0)
            warm2 = cp.tile([1, 16], BF)
            nc.scalar.activation(warm2[:], warm[:], EXP)
            # HAM clock-gate warmup: ~3us of tiny matmuls keep the PE busy
            # through its 4096-cycle activity window while the input DMAs
            # are still in flight, so the real prologue matmuls run warm
            dumm = cp.tile([1, 512], BF)
            nc.vector.memset(dumm[:], 0.0)
            for _ in range(15):
                dw = psp.tile([128, 512], F32, tag="pp", name="dw")
                nc.tensor.matmul(
                    dw[0:16, :], dumm[0:1, 0:16], dumm[0:1, :],
                    start=True, stop=True,
                )

            # ---- weights + inputs ----
            wq2 = cp.tile([128, 4, 128], BF)
            wk2 = cp.tile([128, 4, 128], BF)
            wv2 = cp.tile([128, 4, 128], BF)
            wp2 = cp.tile([HD, 2, 4, 128], BF)

            for t, d in ((wk2, wk2_d), (wq2, wq2_d)):
                for k in range(4):
                    nc.sync.dma_start(t[:, k, :], d[128 * k : 128 * (k + 1), :])
            for k in range(4):
                nc.gpsimd.dma_start(
                    wv2[:, k, :], wv2_d[128 * k : 128 * (k + 1), :]
                )
            nc.gpsimd.dma_start(
                wp2[:].rearrange("p j s c -> p (j s c)"), wp2_d[0:HD, :]
            )

            xT = bp.tile([128, 4, N], BF)
            for k in range(4):
                nc.sync.dma_start(
                    xT[:, k, 0:512], xT_d[128 * k : 128 * (k + 1), 0:512]
                )
            for lo, hi in ((512, 1792), (1792, N)):
                for k in range(4):
                    nc.sync.dma_start(
                        xT[:, k, lo:hi], xT_d[128 * k : 128 * (k + 1), lo:hi]
                    )

            kT = bp.tile([128, N], BF)
            qT = bp.tile([128, N], BF)
            # v1[key, kt, head, hd|1]: 65th column holds ones -> the av
            # matmul accumulates the softmax denominator in out row 64
            v1 = bp.tile([128, NKT, 2, HD + 1], BF)
            nc.vector.memset(v1[:, :, :, HD : HD + 1], 1.0)

            # ---- engine-alternation helpers ----
            exp_acc = [0.0]

            def exp_use_act():
                exp_acc[0] += ACT_FRAC
                if exp_acc[0] >= 1.0:
                    exp_acc[0] -= 1.0
                    return True
                return False

            cp_acc = [0.0]

            def aux_copy(dst, src):
                cp_acc[0] += CPA_FRAC
                if cp_acc[0] >= 1.0:
                    cp_acc[0] -= 1.0
                    nc.scalar.activation(dst, src, COPY)
                else:
                    nc.vector.tensor_copy(dst, src)

            # ---- production units (interleaved with attention) ----
            def prod_k(ci):
                o, n = KCH[ci]
                pp = psp.tile([128, 512], F32, tag="pp", name="pp")
                for k in range(4):
                    nc.tensor.matmul(
                        pp[:, :n],
                        wk2[:, k, :],
                        xT[:, k, o : o + n],
                        start=(k == 0),
                        stop=(k == 3),
                    )
                aux_copy(kT[:, o : o + n], pp[:, :n])

            def prod_q(ci):
                o, n = KCH[ci]
                pp = psp.tile([128, 512], F32, tag="pp", name="pp")
                for k in range(4):
                    nc.tensor.matmul(
                        pp[:, :n],
                        wq2[:, k, :],
                        xT[:, k, o : o + n],
                        start=(k == 0),
                        stop=(k == 3),
                    )
                aux_copy(qT[:, o : o + n], pp[:, :n])

            def prod_v(kt):
                mo, mn = MT[kt]
                pp = psp.tile([128, 512], F32, tag="pp", name="pp")
                for k in range(4):
                    nc.tensor.matmul(
                        pp[:mn, 0:128],
                        xT[:, k, mo : mo + mn],
                        wv2[:, k, :],
                        start=(k == 0),
                        stop=(k == 3),
                    )
                aux_copy(
                    v1[:mn, kt, :, 0:HD],
                    pp[:mn, 0:128].rearrange("p (h c) -> p h c", h=2),
                )

            # production schedule: (ch, slot) -> list of closures
            sched = {}

            def put(ch, slot, fn):
                sched.setdefault((ch, slot), []).append(fn)

            for kt in range(22):  # v tiles 3..24 during ch0
                put(0, kt, (lambda t: lambda: prod_v(t))(kt + 3))
            for c in range(1, 7):  # kT chunks 1..6 during ch0
                put(0, 4 * c - 3, (lambda c_: lambda: prod_k(c_))(c))
            # qT chunks just in time: chunk j first read by query chunk
            # floor(512 j / 392); produce it one chunk earlier
            put(0, 25, lambda: prod_q(1))
            for j, ch in ((2, 1), (3, 2), (4, 3), (5, 4), (6, 5)):
                put(ch, 25, (lambda j_: lambda: prod_q(j_))(j))

            # ---- prologue production: just enough to start ch0 ----
            prod_k(0)
            prod_q(0)
            for kt in range(3):
                prod_v(kt)

            # ---- attention ----
            pending_proj = [None]
            for ch in range(NCH):
                po = psp.tile([128, 2, 512], F32, tag="po", name="po", bufs=1)
                ats = {}
                for slot in range(NKT + DEPTH):
                    if slot < NKT:
                        mo, mn = MT[slot]
                        ps = psp.tile([128, 2, 512], F32, tag="ps", name="ps")
                        for h2 in range(2):
                            hs = slice(64 * h2, 64 * (h2 + 1))
                            nc.tensor.matmul(
                                ps[:mn, h2, :CH],
                                kT[hs, mo : mo + mn],
                                qT[hs, ch * CH : (ch + 1) * CH],
                                start=True,
                                stop=True,
                            )
                        at = atp.tile([128, 2, CH], BF, tag="at", name="at")
                        ats[slot] = at
                        if exp_use_act():
                            nc.scalar.activation(
                                at[:mn, :, :], ps[:mn, :, :CH], EXP
                            )
                        else:
                            nc.vector.tensor_scalar(
                                at[:mn, :, :].bitcast(I16),
                                ps[:mn, :, :CH],
                                EXP_A,
                                EXP_B,
                                MULT,
                                ADD,
                            )
                    kj = slot - DEPTH
                    if kj >= 0:
                        pmo, pmn = MT[kj]
                        pat = ats.pop(kj)
                        for h2 in range(2):
                            nc.tensor.matmul(
                                po[0 : HD + 1, h2, :CH],
                                v1[:pmn, kj, h2, :],
                                pat[:pmn, h2, :],
                                start=(kj == 0),
                                stop=(kj == NKT - 1),
                            )
                    for fn in sched.get((ch, slot), ()):
                        fn()
                    if slot == 2 and pending_proj[0] is not None:
                        pending_proj[0]()
                        pending_proj[0] = None

                # normalize: ONE copy stages the av output (o rows 0-63)
                # into SBUF and a second tiny copy stages the denominator
                # row at base partition 0, releasing the po accumulator;
                # reciprocal / partition-broadcast / multiply then run
                # from SBUF off the po critical path.
                pos = rcp.tile([128, 2, CH], F32, tag="pos", name="pos")
                nc.scalar.activation(pos[0:HD, :, :], po[0:HD, :, :CH], COPY)
                rd = rcp.tile([1, 2, CH], F32, tag="rd", name="rd")
                nc.scalar.activation(rd[0:1, :, :], po[64:65, :, :CH], COPY)
                rc = rcp.tile([1, 2, CH], F32, tag="rc", name="rc")
                nc.vector.reciprocal_approx_fast(
                    out=rc[0:1, :, :], in_=rd[0:1, :, :]
                )
                bcs = rcp.tile([HD, 2, CH], F32, tag="bc", name="bcs")
                nc.gpsimd.partition_broadcast(
                    bcs[0:HD, :, :], rc[0:1, :, :], channels=HD
                )
                outn = rcp.tile([HD, 2, CH], BF, tag="on", name="outn")
                nc.vector.tensor_tensor(
                    outn[0:HD, :, :],
                    pos[0:HD, :, :],
                    bcs[0:HD, :, :],
                    MULT,
                )

                # ---- row-parallel output-projection partial, deferred
                # into the next chunk's early slots (by then the normalize
                # multiply has finished, so the projection matmuls never
                # block the in-order PE queue on it) ----
                def do_proj(ch=ch, outn=outn):
                    for s in range(4):
                        py = psp.tile([128, 512], F32, tag="pp", name="py")
                        for h2 in range(2):
                            nc.tensor.matmul(
                                py[:, :CH],
                                wp2[:, h2, s, :],
                                outn[0:HD, h2, :],
                                start=(h2 == 0),
                                stop=(h2 == 1),
                            )
                        yt = rcp.tile([128, CH], BF, tag="y", name="yt")
                        aux_copy(yt[:, :], py[:, :CH])
                        nc.sync.dma_start(
                            out_d[
                                128 * s : 128 * (s + 1), ch * CH : (ch + 1) * CH
                            ],
                            yt[:, :],
                        )

                pending_proj[0] = do_proj
            pending_proj[0]()

    nc.compile()
    nc.m = get_hw_module(nc.m)
    return nc


def _shard(x, wq, wk, wv, wproj):
    x = np.asarray(x, dtype=np.float32)
    wq = np.asarray(wq, dtype=np.float32)
    wk = np.asarray(wk, dtype=np.float32)
    wv = np.asarray(wv, dtype=np.float32)
    wproj = np.asarray(wproj, dtype=np.float32)

    xT = [np.ascontiguousarray(x[b].T).astype(BF16) for b in range(B)]
    in_maps = []
    for c in range(N_CORES):
        b, hp = c // 4, c % 4
        rows = slice(128 * hp, 128 * (hp + 1))
        wp2 = np.empty((HD, 2, 4, 128), dtype=np.float32)
        for j in range(2):
            for s in range(4):
                wp2[:, j, s, :] = wproj[
                    128 * s : 128 * (s + 1),
                    128 * hp + 64 * j : 128 * hp + 64 * (j + 1),
                ].T
        in_maps.append(
            {
                "xT": xT[b],
                "wq2": np.ascontiguousarray((wq[rows] * SCALE).T).astype(BF16),
                "wk2": np.ascontiguousarray(wk[rows].T).astype(BF16),
                "wv2": np.ascontiguousarray(wv[rows].T).astype(BF16),
                "wp2": np.ascontiguousarray(wp2.reshape(HD, 2 * 4 * 128)).astype(
                    BF16
                ),
            }
        )
    return in_maps


def _unshard(results):
    acc = np.zeros((B, DIM, N), dtype=np.float32)
    for c in range(N_CORES):
        acc[c // 4] += np.asarray(results[c]["out"]).astype(np.float32)
    return np.ascontiguousarray(acc.transpose(0, 2, 1))


def _run(inputs, trace=False):
    from concourse.bass_utils import run_bass_kernel_spmd

    if "nc" not in _CACHE:
        _CACHE["nc"] = _build()
    nc = _CACHE["nc"]
    in_maps = _shard(**inputs)
    res = run_bass_kernel_spmd(
        nc, in_maps, core_ids=list(range(N_CORES)), trace=trace
    )
    return _unshard(res.results), res.exec_time_ns


def kernel(**inputs) -> np.ndarray:
    return _run(inputs, trace=False)[0]
